# revision 1
# baseline (speedup 1.0000x reference)
"""Depthwise-masked 3x3 conv (eye-masked dense conv) on 8 TRN2 NeuronCores.

Problem: x (2,16,256,64,64) fp32, W (256,256,3,3) fp32; the reference masks W
with eye(C) so only W[c,c,:,:] survives -> depthwise 3x3 "same" conv.

Strategy (per core; data-parallel over the 32 (s,b) samples -> 4 samples/core):
  - channels on partitions: work tile = (sample, channel-block of 128) ->
    x tile [128, 64, 64]; 8 work tiles per core, split per half-tile (32 rows)
    between the PE and DVE paths to balance engine load.
  - PE path: 9 taps as diagonal-stationary matmuls in fp32r (1 cycle/row),
    accumulating in PSUM per 512-element bank chunk (8 output rows). Both
    H and W boundaries handled by row/column-clipped access patterns; the
    x and wd DRAM tensors are declared float32r (same bytes as fp32) so no
    on-chip cast/repack is needed.
  - DVE path: fused scalar_tensor_tensor (out = x_shift * w_c + out) with
    per-partition scalars, clipped views; ScalarE does the center tap and
    PSUM eviction.
"""

import os
from contextlib import ExitStack

import numpy as np

import concourse.bass as bass
import concourse.tile as tile
from concourse import bacc, mybir
from concourse.bass_utils import run_bass_kernel_spmd

S, B, C, H, W_SP = 2, 16, 256, 64, 64
N_CORES = 8
N_SAMPLES = S * B                      # 32
SPC = N_SAMPLES // N_CORES             # 4 samples per core
NBLK = C // 128                        # 2 channel blocks
N_TILES = SPC * NBLK                   # 8 work tiles per core
WPAD = W_SP + 2                        # 66: zero col, 64 data cols, zero col
ROWS_PER_CHUNK = 8                     # 512 fp32 = one PSUM bank
HALF_CHUNKS = 4                        # chunks per half tile (4 banks)
HALF_ROWS = HALF_CHUNKS * ROWS_PER_CHUNK  # 32
HSPLIT = HALF_ROWS + 2                 # xc half-DMA split row

# center tap first: start=True matmul must cover the full bank
TAPS = [(0, 0), (-1, -1), (-1, 0), (-1, 1), (0, -1), (0, 1), (1, -1), (1, 0), (1, 1)]

# half-tiles routed to the DVE path: (tile, half) pairs
_DVE_HALVES_DEFAULT = "0.1,3.0,5.0,6.0,6.1"
DVE_HALVES = frozenset(
    tuple(int(v) for v in p.split("."))
    for p in os.environ.get("KERNEL_DVE_HALVES", _DVE_HALVES_DEFAULT).split(",")
    if p
)

F32 = mybir.dt.float32
F32R = mybir.dt.float32r


def _emit_pe_half(nc, pools, g, half, xc, out_d, wd_sb):
    """PE path for rows [32*half, 32*half+32) of work tile g."""
    _, psum_pool, osb_pool, _ = pools
    psum = psum_pool.tile([128, HALF_CHUNKS, ROWS_PER_CHUNK, W_SP], F32, tag="psum")
    for t, (dh, dw) in enumerate(TAPS):
        lhsT = wd_sb[:, ((g % NBLK) * 9 + t) * 128:((g % NBLK) * 9 + t + 1) * 128]
        for q in range(HALF_CHUNKS):
            h0 = half * HALF_ROWS + q * ROWS_PER_CHUNK
            a = max(h0, -dh)                      # first valid output row
            b = min(h0 + ROWS_PER_CHUNK, H - dh)  # one past last valid row
            rhs = xc[:, a + dh:b + dh, 1 + dw:1 + dw + W_SP]
            out_ap = psum[:, q, a - h0:b - h0, :]
            nc.tensor.matmul(out_ap, lhsT, rhs,
                             start=(t == 0), stop=(t == len(TAPS) - 1))
    osb = osb_pool.tile([128, HALF_CHUNKS * 512], F32, tag="osb")
    nc.scalar.copy(osb[:], psum[:, :, :, :])
    nc.sync.dma_start(
        out_d[g * 128:(g + 1) * 128, half * HALF_ROWS:(half + 1) * HALF_ROWS, :],
        osb[:],
    )


def _emit_dve_half(nc, pools, g, half, xc, out_d, wv_sb):
    """DVE path for rows [32*half, 32*half+32) of work tile g."""
    _, _, _, odve_pool = pools
    out = odve_pool.tile([128, HALF_ROWS, W_SP], F32, tag="odve")
    xcf = xc[:].bitcast(F32)
    cb = g % NBLK
    r0 = half * HALF_ROWS
    for t, (dh, dw) in enumerate(TAPS):
        wv = wv_sb[:, cb * 9 + t:cb * 9 + t + 1]
        if t == 0:
            nc.vector.tensor_scalar(
                out[:], xcf[:, r0:r0 + HALF_ROWS, :], wv, None, mybir.AluOpType.mult)
        else:
            # output rows (within the half) whose shifted source row exists
            oa = max(0, -(r0 + dh))
            ob = min(HALF_ROWS, H - dh - r0)
            oc, od = max(0, -dw), W_SP - max(0, dw)
            out_v = out[:, oa:ob, oc:od]
            in_v = xcf[:, r0 + oa + dh:r0 + ob + dh, oc + dw:od + dw]
            nc.vector.scalar_tensor_tensor(
                out_v, in_v, wv, out_v,
                op0=mybir.AluOpType.mult, op1=mybir.AluOpType.add,
            )
    nc.sync.dma_start(
        out_d[g * 128:(g + 1) * 128, r0:r0 + HALF_ROWS, :], out[:])


def _build_program(dve_halves):
    nc = bacc.Bacc("TRN2", target_bir_lowering=False, debug=False)
    # x and wd carry fp32 bytes but are declared float32r so the PE can
    # consume them directly (PE truncates the extra mantissa bits).
    x_d = nc.dram_tensor("x", [SPC * C, H, W_SP], F32R, kind="ExternalInput").ap()
    wd_d = nc.dram_tensor("wd", [128, NBLK * 9 * 128], F32R, kind="ExternalInput").ap()
    wv_d = nc.dram_tensor("wv", [128, NBLK * 9], F32, kind="ExternalInput").ap()
    out_d = nc.dram_tensor("out", [SPC * C, H, W_SP], F32, kind="ExternalOutput").ap()

    with tile.TileContext(nc) as tc:
        with ExitStack() as ctx:
            const_pool = ctx.enter_context(tc.tile_pool(name="const", bufs=1))
            wd_sb = const_pool.tile([128, NBLK * 9 * 128], F32R)
            nc.sync.dma_start(wd_sb[:], wd_d[:])
            wv_sb = const_pool.tile([128, NBLK * 9], F32)
            nc.sync.dma_start(wv_sb[:], wv_d[:])
            zf_sb = const_pool.tile([128, H, 1], F32)
            nc.vector.memset(zf_sb[:], 0.0)
            zero_sb = const_pool.tile([128, H, 1], F32R)
            nc.vector.tensor_copy(zero_sb[:], zf_sb[:])  # fp32r zeros for pad cols

            xc_pool = ctx.enter_context(tc.tile_pool(name="xc", bufs=3))
            xp_pool = ctx.enter_context(tc.tile_pool(name="xp", bufs=3))
            psum_pool = ctx.enter_context(tc.tile_pool(name="psum", bufs=2, space="PSUM"))
            osb_pool = ctx.enter_context(tc.tile_pool(name="osb", bufs=4))
            odve_pool = ctx.enter_context(tc.tile_pool(name="odve", bufs=4))
            pools = (xc_pool, psum_pool, osb_pool, odve_pool)

            for g in range(N_TILES):
                halves = [(g, 0) in dve_halves, (g, 1) in dve_halves]
                xc = xc_pool.tile([128, H, W_SP], F32R, tag="xc")
                # contiguous load, split in two so dependent work starts
                # early; stagger issue so early tiles get full DMA bandwidth
                with tc.tile_wait_until(g * 0.008):
                    nc.sync.dma_start(xc[:, 0:HSPLIT, :],
                                      x_d[g * 128:(g + 1) * 128, 0:HSPLIT, :])
                    nc.sync.dma_start(xc[:, HSPLIT:H, :],
                                      x_d[g * 128:(g + 1) * 128, HSPLIT:H, :])

                xp = None
                if not all(halves):
                    # padded tile for PE halves (plain f32r copies, no cast)
                    xp = xp_pool.tile([128, H, WPAD], F32R, tag="xp")
                    nc.scalar.copy(xp[:, :, 0:1], zero_sb[:])
                    nc.scalar.copy(xp[:, :, WPAD - 1:WPAD], zero_sb[:])
                    nc.scalar.copy(xp[:, 0:HSPLIT, 1:1 + W_SP], xc[:, 0:HSPLIT, :])
                    nc.scalar.copy(xp[:, HSPLIT:H, 1:1 + W_SP], xc[:, HSPLIT:H, :])

                for half in range(2):
                    if halves[half]:
                        _emit_dve_half(nc, pools, g, half, xc, out_d, wv_sb)
                    else:
                        _emit_pe_half(nc, pools, g, half, xp, out_d, wd_sb)
    nc.compile()
    return nc


_prog_cache = {}


def _get_program():
    key = DVE_HALVES
    if key not in _prog_cache:
        _prog_cache[key] = _build_program(key)
    return _prog_cache[key]


def _host_weights(W):
    wdiag = W[np.arange(C), np.arange(C)]          # [256, 3, 3]
    wd_host = np.zeros((128, NBLK * 9, 128), dtype=np.float32)
    wv_host = np.zeros((128, NBLK * 9), dtype=np.float32)
    r = np.arange(128)
    for cb in range(NBLK):
        for t, (dh, dw) in enumerate(TAPS):
            wd_host[r, cb * 9 + t, r] = wdiag[cb * 128 + r, dh + 1, dw + 1]
            wv_host[r, cb * 9 + t] = wdiag[cb * 128 + r, dh + 1, dw + 1]
    return wd_host.reshape(128, NBLK * 9 * 128), wv_host


def _in_maps(x, W):
    wd_host, wv_host = _host_weights(W)
    xs = x.reshape(N_SAMPLES, C, H, W_SP)
    return [
        {
            "x": np.ascontiguousarray(xs[i * SPC:(i + 1) * SPC]).reshape(SPC * C, H, W_SP),
            "wd": wd_host,
            "wv": wv_host,
        }
        for i in range(N_CORES)
    ]


def kernel(x: np.ndarray, W: np.ndarray) -> np.ndarray:
    x = np.ascontiguousarray(x, dtype=np.float32)
    W = np.ascontiguousarray(W, dtype=np.float32)
    assert x.shape == (S, B, C, H, W_SP)
    assert W.shape == (C, C, 3, 3)

    nc = _get_program()
    res = run_bass_kernel_spmd(nc, _in_maps(x, W), core_ids=list(range(N_CORES)))
    out = np.concatenate(
        [res.results[i]["out"].reshape(SPC, C, H, W_SP) for i in range(N_CORES)], axis=0
    )
    return out.reshape(S, B, C, H, W_SP)



# revision 2
# speedup vs baseline: 2.0295x; 2.0295x over previous
"""Depthwise-masked 3x3 conv (eye-masked dense conv) on 8 TRN2 NeuronCores.

Problem: x (2,16,256,64,64) fp32, W (256,256,3,3) fp32; the reference masks W
with eye(C) so only W[c,c,:,:] survives -> depthwise 3x3 "same" conv.

Strategy (v2): channel-sharded across cores (32 ch/core, all 32 samples),
fp16 end-to-end on the wire (rel-err budget is 2e-2; fp16 keeps it ~1e-3),
and the whole conv runs on the PE as banded-Toeplitz matmuls:

  - partitions = (2 channels) x (64 H rows); lhsT [128,128] is block-diagonal
    with one 64x64 3-band Toeplitz block per channel, so a single matmul
    computes the full 3-tap H-convolution for 2 channels at once
    (384 useful MACs/cycle vs 128 for the diagonal-weights scheme).
  - 3 matmul passes per channel pair (dw = 0,-1,+1), W-boundary handled by
    column-clipped rhs/out access patterns, H-boundary by the band structure.
  - free dim = (32 samples x 64 w) = 2048 f32 PSUM = 4 banks, chunked into
    4 bank-sized matmuls of N=512; weights are reused across all 32 samples
    so only 16 pairs x 3 dw = 48 small lhsT loads per core.
  - eviction PSUM->SBUF casts to fp16, alternating ScalarE/VectorE per pair;
    host does all layout (pair packing, fp16 cast, final fp32 cast).

Per-core traffic: 8.4 MB in + 8.4 MB out + 1.5 MB weights (fp16) ~= 48 us at
358 GB/s; PE compute ~= 42 us; both near the ridge.
"""

from contextlib import ExitStack

import numpy as np

import concourse.bass as bass
import concourse.tile as tile
from concourse import bacc, mybir
from concourse.bass_utils import run_bass_kernel_spmd

S, B, C, H, W_SP = 2, 16, 256, 64, 64
N_CORES = 8
NS = S * B                  # 32 samples (all on every core)
CPC = C // N_CORES          # 32 channels per core
NPAIR = CPC // 2            # 16 channel pairs per core
NCHUNK = 4                  # PSUM bank chunks per pair (512 f32 each)
SCH = NS // NCHUNK          # 8 samples per chunk
DWS = [0, -1, 1]            # dw=0 first: start=True must cover the full bank

F16 = mybir.dt.float16
F32 = mybir.dt.float32


def _build_program():
    nc = bacc.Bacc("TRN2", target_bir_lowering=False, debug=False)
    x_d = nc.dram_tensor("x", [NPAIR * 128, NS, W_SP], F16, kind="ExternalInput").ap()
    wt_d = nc.dram_tensor("wt", [128, NPAIR * 3 * 128], F16, kind="ExternalInput").ap()
    out_d = nc.dram_tensor("out", [NPAIR * 128, NS * W_SP], F16, kind="ExternalOutput").ap()

    with tile.TileContext(nc) as tc:
        with ExitStack() as ctx:
            const_pool = ctx.enter_context(tc.tile_pool(name="const", bufs=1))
            wsb = const_pool.tile([128, NPAIR * 3 * 128], F16)
            nc.sync.dma_start(wsb[:], wt_d[:])

            xt_pool = ctx.enter_context(tc.tile_pool(name="xt", bufs=3))
            psum_pool = ctx.enter_context(tc.tile_pool(name="psum", bufs=2, space="PSUM"))
            osb_pool = ctx.enter_context(tc.tile_pool(name="osb", bufs=3))

            for k in range(NPAIR):
                xt = xt_pool.tile([128, NS, W_SP], F16, tag="xt")
                r0, r1 = k * 128, (k + 1) * 128
                nc.sync.dma_start(xt[:, 0:NS // 2, :], x_d[r0:r1, 0:NS // 2, :])
                nc.sync.dma_start(xt[:, NS // 2:NS, :], x_d[r0:r1, NS // 2:NS, :])

                pt = psum_pool.tile([128, NCHUNK, SCH, W_SP], F32, tag="pt")
                for j, dw in enumerate(DWS):
                    lhsT = wsb[:, (k * 3 + j) * 128:(k * 3 + j + 1) * 128]
                    for q in range(NCHUNK):
                        s0 = q * SCH
                        if dw == 0:
                            out_ap = pt[:, q, :, :]
                            rhs = xt[:, s0:s0 + SCH, :]
                        elif dw == -1:
                            out_ap = pt[:, q, :, 1:W_SP]
                            rhs = xt[:, s0:s0 + SCH, 0:W_SP - 1]
                        else:
                            out_ap = pt[:, q, :, 0:W_SP - 1]
                            rhs = xt[:, s0:s0 + SCH, 1:W_SP]
                        nc.tensor.matmul(out_ap, lhsT, rhs,
                                         start=(j == 0), stop=(j == 2))

                ob = osb_pool.tile([128, NCHUNK * SCH * W_SP], F16, tag="ob")
                if k % 2 == 0:
                    nc.scalar.copy(ob[:], pt[:, :, :, :])
                else:
                    nc.vector.tensor_copy(ob[:], pt[:, :, :, :])
                nc.sync.dma_start(out_d[r0:r1, :], ob[:])
    nc.compile()
    return nc


_prog_cache = {}


def _get_program():
    if "p" not in _prog_cache:
        _prog_cache["p"] = _build_program()
    return _prog_cache["p"]


def _in_maps(x, W):
    wdiag = W[np.arange(C), np.arange(C)].astype(np.float32)   # [C,3,3]
    xs = x.reshape(NS, C, H, W_SP).astype(np.float16)
    eye = {d: np.eye(H, k=-d, dtype=np.float32) for d in (-1, 0, 1)}
    in_maps = []
    for core in range(N_CORES):
        c0 = core * CPC
        # x: [pair, (2ch x 64h) partitions, sample, w]
        A = xs[:, c0:c0 + CPC].transpose(1, 2, 0, 3)           # [32c, 64h, 32s, 64w]
        X = np.ascontiguousarray(
            A.reshape(2, NPAIR, H, NS, W_SP).transpose(1, 0, 2, 3, 4)
        ).reshape(NPAIR * 128, NS, W_SP)
        # weights: per (pair, dw) a block-diagonal pair of 3-band Toeplitz
        # matrices; lhsT[p, o] = w[c(o), (p-o)+1, dw+1] for |p-o| <= 1
        wt = np.zeros((NPAIR, 3, 128, 128), dtype=np.float32)
        for k in range(NPAIR):
            for j, dw in enumerate(DWS):
                for half in range(2):
                    c = c0 + k + 16 * half
                    T = (wdiag[c, 0, dw + 1] * eye[-1]
                         + wdiag[c, 1, dw + 1] * eye[0]
                         + wdiag[c, 2, dw + 1] * eye[1])
                    h0 = 64 * half
                    wt[k, j, h0:h0 + 64, h0:h0 + 64] = T
        wt_host = np.ascontiguousarray(
            wt.transpose(2, 0, 1, 3)
        ).reshape(128, NPAIR * 3 * 128).astype(np.float16)
        in_maps.append({"x": X, "wt": wt_host})
    return in_maps


def kernel(x: np.ndarray, W: np.ndarray) -> np.ndarray:
    x = np.ascontiguousarray(x, dtype=np.float32)
    W = np.ascontiguousarray(W, dtype=np.float32)
    assert x.shape == (S, B, C, H, W_SP)
    assert W.shape == (C, C, 3, 3)

    nc = _get_program()
    res = run_bass_kernel_spmd(nc, _in_maps(x, W), core_ids=list(range(N_CORES)))
    outs = []
    for core in range(N_CORES):
        oc = res.results[core]["out"].reshape(NPAIR, 2, H, NS, W_SP)
        outs.append(oc.transpose(3, 1, 0, 2, 4).reshape(NS, CPC, H, W_SP))
    out = np.concatenate(outs, axis=1).astype(np.float32)
    return out.reshape(S, B, C, H, W_SP)


# revision 4
# speedup vs baseline: 2.5170x; 1.2402x over previous
"""Depthwise-masked 3x3 conv (eye-masked dense conv) on 8 TRN2 NeuronCores.

Problem: x (2,16,256,64,64) fp32, W (256,256,3,3) fp32; the reference masks W
with eye(C) so only W[c,c,:,:] survives -> depthwise 3x3 "same" conv.

Strategy (v2): channel-sharded across cores (32 ch/core, all 32 samples),
fp16 end-to-end on the wire (rel-err budget is 2e-2; fp16 keeps it ~1e-3),
and the whole conv runs on the PE as banded-Toeplitz matmuls:

  - partitions = (2 channels) x (64 H rows); lhsT [128,128] is block-diagonal
    with one 64x64 3-band Toeplitz block per channel, so a single matmul
    computes the full 3-tap H-convolution for 2 channels at once
    (384 useful MACs/cycle vs 128 for the diagonal-weights scheme).
  - 3 matmul passes per channel pair (dw = 0,-1,+1), W-boundary handled by
    column-clipped rhs/out access patterns, H-boundary by the band structure.
  - free dim = (32 samples x 64 w) = 2048 f32 PSUM = 4 banks, chunked into
    4 bank-sized matmuls of N=512; weights are reused across all 32 samples
    so only 16 pairs x 3 dw = 48 small lhsT loads per core.
  - eviction PSUM->SBUF casts to fp16, alternating ScalarE/VectorE per pair;
    host does all layout (pair packing, fp16 cast, final fp32 cast).

Per-core traffic: 8.4 MB in + 8.4 MB out + 1.5 MB weights (fp16) ~= 48 us at
358 GB/s; PE compute ~= 42 us; both near the ridge.
"""

from contextlib import ExitStack

import numpy as np

import concourse.bass as bass
import concourse.tile as tile
from concourse import bacc, mybir
from concourse.bass_utils import run_bass_kernel_spmd

S, B, C, H, W_SP = 2, 16, 256, 64, 64
N_CORES = 8
NS = S * B                  # 32 samples (all on every core)
CPC = C // N_CORES          # 32 channels per core
NPAIR = CPC // 2            # 16 channel pairs per core
NCHUNK = 4                  # PSUM bank chunks per pair (512 f32 each)
SCH = NS // NCHUNK          # 8 samples per chunk
DWS = [0, -1, 1]            # dw=0 first: start=True must cover the full bank

F16 = mybir.dt.float16
F32 = mybir.dt.float32


def _build_program():
    nc = bacc.Bacc("TRN2", target_bir_lowering=False, debug=False)
    x_d = nc.dram_tensor("x", [NPAIR * 128, NS, W_SP], F16, kind="ExternalInput").ap()
    wt_d = nc.dram_tensor("wt", [128, NPAIR * 3 * 128], F16, kind="ExternalInput").ap()
    out_d = nc.dram_tensor("out", [NPAIR * 128, NS * W_SP], F16, kind="ExternalOutput").ap()

    WCH = NPAIR // 4  # weight DMA chunk: 4 pairs each
    with tile.TileContext(nc) as tc:
        with ExitStack() as ctx:
            const_pool = ctx.enter_context(tc.tile_pool(name="const", bufs=1))
            wsb = const_pool.tile([128, NPAIR * 3 * 128], F16)

            xt_pool = ctx.enter_context(tc.tile_pool(name="xt", bufs=4))
            psum_pool = ctx.enter_context(tc.tile_pool(name="psum", bufs=4, space="PSUM"))
            osb_pool = ctx.enter_context(tc.tile_pool(name="osb", bufs=3))

            # weight chunk 0 first so pair 0 can start as soon as its x lands
            wslice = 4 * 3 * 128
            nc.sync.dma_start(wsb[:, 0:wslice], wt_d[:, 0:wslice])

            xts = []
            for k in range(NPAIR):
                xt = xt_pool.tile([128, NS, W_SP], F16, tag="xt")
                r0, r1 = k * 128, (k + 1) * 128
                nc.sync.dma_start(xt[:], x_d[r0:r1, :, :])
                xts.append(xt)
                if k in (0, 1, 2):  # remaining weight chunks interleaved early
                    c = k + 1
                    nc.sync.dma_start(wsb[:, c * wslice:(c + 1) * wslice],
                                      wt_d[:, c * wslice:(c + 1) * wslice])

            for k in range(NPAIR):
                xt = xts[k]
                r0, r1 = k * 128, (k + 1) * 128
                # two half-pair PSUM tiles (2 banks each) for finer pipelining
                pt_a = psum_pool.tile([128, 2, SCH, W_SP], F32, tag="pt")
                pt_b = psum_pool.tile([128, 2, SCH, W_SP], F32, tag="pt")
                pts = [pt_a, pt_b]
                for j, dw in enumerate(DWS):
                    lhsT = wsb[:, (k * 3 + j) * 128:(k * 3 + j + 1) * 128]
                    for q in range(NCHUNK):
                        pt = pts[q // 2]
                        s0 = q * SCH
                        if dw == 0:
                            out_ap = pt[:, q % 2, :, :]
                            rhs = xt[:, s0:s0 + SCH, :]
                        elif dw == -1:
                            out_ap = pt[:, q % 2, :, 1:W_SP]
                            rhs = xt[:, s0:s0 + SCH, 0:W_SP - 1]
                        else:
                            out_ap = pt[:, q % 2, :, 0:W_SP - 1]
                            rhs = xt[:, s0:s0 + SCH, 1:W_SP]
                        nc.tensor.matmul(out_ap, lhsT, rhs,
                                         start=(j == 0), stop=(j == 2))

                ob = osb_pool.tile([128, NCHUNK * SCH * W_SP], F16, tag="ob")
                hfd = 2 * SCH * W_SP
                for half in range(2):
                    dst = ob[:, half * hfd:(half + 1) * hfd]
                    if (k + half) % 2 == 0:
                        nc.scalar.copy(dst, pts[half][:, :, :, :])
                    else:
                        nc.vector.tensor_copy(dst, pts[half][:, :, :, :])
                nc.scalar.dma_start(out_d[r0:r1, :], ob[:])
    nc.compile()
    return nc


_prog_cache = {}


def _get_program():
    if "p" not in _prog_cache:
        _prog_cache["p"] = _build_program()
    return _prog_cache["p"]


def _in_maps(x, W):
    wdiag = W[np.arange(C), np.arange(C)].astype(np.float32)   # [C,3,3]
    xs = x.reshape(NS, C, H, W_SP).astype(np.float16)
    eye = {d: np.eye(H, k=-d, dtype=np.float32) for d in (-1, 0, 1)}
    in_maps = []
    for core in range(N_CORES):
        c0 = core * CPC
        # x: [pair, (2ch x 64h) partitions, sample, w]
        A = xs[:, c0:c0 + CPC].transpose(1, 2, 0, 3)           # [32c, 64h, 32s, 64w]
        X = np.ascontiguousarray(
            A.reshape(2, NPAIR, H, NS, W_SP).transpose(1, 0, 2, 3, 4)
        ).reshape(NPAIR * 128, NS, W_SP)
        # weights: per (pair, dw) a block-diagonal pair of 3-band Toeplitz
        # matrices; lhsT[p, o] = w[c(o), (p-o)+1, dw+1] for |p-o| <= 1
        wt = np.zeros((NPAIR, 3, 128, 128), dtype=np.float32)
        for k in range(NPAIR):
            for j, dw in enumerate(DWS):
                for half in range(2):
                    c = c0 + k + 16 * half
                    T = (wdiag[c, 0, dw + 1] * eye[-1]
                         + wdiag[c, 1, dw + 1] * eye[0]
                         + wdiag[c, 2, dw + 1] * eye[1])
                    h0 = 64 * half
                    wt[k, j, h0:h0 + 64, h0:h0 + 64] = T
        wt_host = np.ascontiguousarray(
            wt.transpose(2, 0, 1, 3)
        ).reshape(128, NPAIR * 3 * 128).astype(np.float16)
        in_maps.append({"x": X, "wt": wt_host})
    return in_maps


def kernel(x: np.ndarray, W: np.ndarray) -> np.ndarray:
    x = np.ascontiguousarray(x, dtype=np.float32)
    W = np.ascontiguousarray(W, dtype=np.float32)
    assert x.shape == (S, B, C, H, W_SP)
    assert W.shape == (C, C, 3, 3)

    nc = _get_program()
    res = run_bass_kernel_spmd(nc, _in_maps(x, W), core_ids=list(range(N_CORES)))
    outs = []
    for core in range(N_CORES):
        oc = res.results[core]["out"].reshape(NPAIR, 2, H, NS, W_SP)
        outs.append(oc.transpose(3, 1, 0, 2, 4).reshape(NS, CPC, H, W_SP))
    out = np.concatenate(outs, axis=1).astype(np.float32)
    return out.reshape(S, B, C, H, W_SP)
